# revision 34
# baseline (speedup 1.0000x reference)
"""FCOS detection-head decode kernel for Trainium2 (Bass/Tile), 8-core data-parallel.

Problem: nB=32, nH=nW=128, nCls=80, stride=8.
  p_ltrb  = exp(t_ltrb) * 8
  p_xywh  = ltrb->xywh against the 8-px grid
  cls_idx = argmax_c cls_logits  (== argmax of sigmoid(cls_logits), sigmoid monotone;
            verified bit-exact vs the reference on the fixed dataset incl. ties)
  confs   = sqrt(sigmoid(center) * sigmoid(max_logit))

Sharding: batch dim 32 -> 4 batches per core, 8 cores SPMD (no collectives).

Per-core pipeline (cells = 4*128*128 = 65536, cell-major layout, K cells/partition):
  DVE : m   = grouped reduce_max over classes            [128, K, 80] -> [128, K]
  DVE : eq  = (x >= m) as bf16 0/1                       (TT is_ge, broadcast m)
  DVE : s   = eq * rev (rev = 79-c, bf16, 2x mode)
  DVE : mx  = grouped max(s) via bf16 2x halving tree    -> idx = 79-mx (first occurrence)
  ACT : sigmoid(m), sigmoid(center), sqrt; exp for boxes
"""

import os
import numpy as np
import ml_dtypes

import concourse.bass as bass
import concourse.bacc as bacc
import concourse.tile as tile
from concourse import mybir
from concourse.bass_utils import run_bass_kernel_spmd

F32 = mybir.dt.float32
BF16 = mybir.dt.bfloat16
I32 = mybir.dt.int32
ALU = mybir.AluOpType
AF = mybir.ActivationFunctionType
AXX = mybir.AxisListType.X

N_CORES = 8
B_PER = 4                      # batches per core
H = W = 128
NCLS = 80
CELLS = B_PER * H * W          # 65536 cells per core
K = 64                         # cells per partition per cls tile
TCELLS = 128 * K               # cells per tile
NT = CELLS // TCELLS           # number of cls tiles
STRIDE = 8.0
LN4 = float(np.log(4.0))
LN8 = float(np.log(8.0))
KACT = 54  # k-slices per full tile whose compare runs on ACT (Sign) instead of DVE

_prog_cache = {}


def _build_program():
    nc = bacc.Bacc("TRN2", debug=False)

    cls_t = nc.dram_tensor("cls", [CELLS, NCLS], F32, kind="ExternalInput")
    ctr_t = nc.dram_tensor("ctr", [CELLS], F32, kind="ExternalInput")
    ltrb_t = nc.dram_tensor("ltrb", [B_PER, H, W, 4], F32, kind="ExternalInput")
    xywh_t = nc.dram_tensor("xywh", [B_PER, H, W, 4], F32, kind="ExternalOutput")
    idx_t = nc.dram_tensor("idx", [CELLS], I32, kind="ExternalOutput")
    conf_t = nc.dram_tensor("conf", [CELLS], F32, kind="ExternalOutput")

    # Constants embedded in the NEFF. rev = (79-c)/128: exact in bf16 (k/128,
    # k < 128), so every value in the extract chain stays exact.
    rev_np = np.broadcast_to(
        ((79 - np.arange(NCLS)) / 128.0).astype(ml_dtypes.bfloat16), (128, NCLS)
    )
    xg_np = np.broadcast_to((np.arange(W) * STRIDE + STRIDE / 2).astype(np.float32), (128, W))
    yg_np = (np.arange(128) * STRIDE + STRIDE / 2).astype(np.float32).reshape(128, 1)
    rev_d = nc.inline_tensor(np.ascontiguousarray(rev_np), "rev_const")
    xg_d = nc.inline_tensor(np.ascontiguousarray(xg_np), "xg_const")
    yg_d = nc.inline_tensor(yg_np, "yg_const")

    from contextlib import ExitStack

    with ExitStack() as ctx:
        tc = ctx.enter_context(tile.TileContext(nc))
        consts = ctx.enter_context(tc.tile_pool(name="consts", bufs=1))
        xp = ctx.enter_context(tc.tile_pool(name="xp", bufs=3))
        eqp = ctx.enter_context(tc.tile_pool(name="eqp", bufs=2))
        sp = ctx.enter_context(tc.tile_pool(name="sp", bufs=3))
        zp = ctx.enter_context(tc.tile_pool(name="zp", bufs=3))
        tp = ctx.enter_context(tc.tile_pool(name="tp", bufs=2))
        smalls = ctx.enter_context(tc.tile_pool(name="smalls", bufs=3))
        prodp = ctx.enter_context(tc.tile_pool(name="prodp", bufs=1))
        boxp = ctx.enter_context(tc.tile_pool(name="boxp", bufs=1))

        rev_sb = consts.tile([128, NCLS], BF16)
        nc.sync.dma_start(out=rev_sb[:], in_=rev_d.ap())
        xg_sb = consts.tile([128, W], F32)
        yg_sb = consts.tile([128, 1], F32)
        ln4_sb = consts.tile([128, 1], F32)
        ln8_sb = consts.tile([128, 1], F32)

        def emit_box_consts():
            nc.sync.dma_start(out=xg_sb[:], in_=xg_d.ap())
            nc.sync.dma_start(out=yg_sb[:], in_=yg_d.ap())
            nc.vector.memset(ln4_sb[:], LN4)
            nc.vector.memset(ln8_sb[:], LN8)

        cls_f = cls_t.ap()
        ctr_f = ctr_t.ap()
        idx_f = idx_t.ap()
        conf_f = conf_t.ap()

        def emit_boxes():
            # Box path: one tile for all batches, [128 part = h, (b, w, 4)] layout.
            lt = boxp.tile([128, B_PER, W, 4], F32, tag="lt")
            nc.sync.dma_start(
                out=lt[:], in_=ltrb_t.ap().rearrange("b h w c -> h b w c")
            )
            a4 = boxp.tile([128, B_PER, W, 4], F32, tag="a4")
            nc.scalar.activation(a4[:], lt[:], AF.Exp, bias=ln4_sb[:])  # 4*exp(t)
            a8 = boxp.tile([128, B_PER, W, 4], F32, tag="a8")
            nc.scalar.activation(a8[:], lt[:], AF.Exp, bias=ln8_sb[:])  # 8*exp(t)

            xy = boxp.tile([128, B_PER, W, 4], F32, tag="xy")
            tmp = boxp.tile([128, B_PER, W], F32, tag="btmp")
            # cx = xg + (a4_r - a4_l)
            nc.vector.tensor_tensor(tmp[:], a4[:, :, :, 2], a4[:, :, :, 0], ALU.subtract)
            nc.vector.tensor_tensor(
                xy[:, :, :, 0], tmp[:],
                xg_sb[:, None, :].to_broadcast([128, B_PER, W]), ALU.add,
            )
            # cy = (a4_b + yg) - a4_t   (one fused scalar_tensor_tensor)
            nc.vector.scalar_tensor_tensor(
                xy[:, :, :, 1], a4[:, :, :, 3], yg_sb[:], a4[:, :, :, 1],
                op0=ALU.add, op1=ALU.subtract,
            )
            # w = a8_l + a8_r ; h = a8_t + a8_b
            nc.vector.tensor_tensor(xy[:, :, :, 2], a8[:, :, :, 0], a8[:, :, :, 2], ALU.add)
            nc.vector.tensor_tensor(xy[:, :, :, 3], a8[:, :, :, 1], a8[:, :, :, 3], ALU.add)
            nc.sync.dma_start(
                out=xywh_t.ap().rearrange("b h w c -> h b w c"), in_=xy[:]
            )

        # Ramp-up schedule: small first tiles so DVE starts quickly, then
        # full-size tiles. Sum of Ks must equal CELLS // 128.
        KS = [8, 8, 16, 32] + [K] * ((CELLS // 128 - 128) // K) + [32, 32]
        assert sum(KS) == CELLS // 128

        pend_conf = []
        pend_sqrt = []
        c0 = 0
        for t, Kt in enumerate(KS):
            n = 128 * Kt
            cls_vt = cls_f[c0 : c0 + n].rearrange("(p k) c -> p k c", k=Kt)
            ctr_vt = ctr_f[c0 : c0 + n].rearrange("(p k) -> p k", k=Kt)
            idx_vt = idx_f[c0 : c0 + n].rearrange("(p k) -> p k", k=Kt)
            conf_vt = conf_f[c0 : c0 + n].rearrange("(p k) -> p k", k=Kt)
            c0 += n

            x = xp.tile([128, Kt, NCLS], F32, tag="x")
            nc.sync.dma_start(out=x[:], in_=cls_vt)

            # m and center logits share one tile so one Sigmoid covers both.
            # Per-tile tag: mc stays alive until the batched sigmoid flush.
            mc = prodp.tile([128, 2 * Kt], F32, tag=f"mc{t}")
            m = mc[:, 0:Kt]
            nc.vector.reduce_max(m, x[:], axis=AXX)
            nc.sync.dma_start(out=mc[:, Kt : 2 * Kt], in_=ctr_vt)

            s = sp.tile([128, Kt, NCLS], BF16, tag="s")

            # Split the compare between ACT (Sign with per-partition -m bias,
            # one k-slice per instruction) and DVE (is_ge TT). ACT is idle;
            # the ACT share's masked-rev product fuses into one 4x-mode STT.
            ka = KACT if Kt == K else 0
            if ka:
                mneg = smalls.tile([128, Kt], F32, tag="mneg")
                nc.vector.tensor_scalar(mneg[:], m, -1.0, None, op0=ALU.mult)
                z = zp.tile([128, ka, NCLS], BF16, tag="z")
                for kk in range(ka):
                    # z = sign(x - m) in {-1, 0}
                    nc.scalar.sign(z[:, kk, :], x[:, kk, :], bias=mneg[:, kk : kk + 1])
                # s = z + rev: achiever -> rev >= 0, else rev-1 < 0 (all exact bf16)
                nc.vector.tensor_tensor(
                    s[:, 0:ka, :], z[:],
                    rev_sb[:, None, :].to_broadcast([128, ka, NCLS]), ALU.add,
                )

            kd = Kt - ka
            if kd:
                eq = eqp.tile([128, kd, NCLS], BF16, tag="eq")
                nc.vector.tensor_tensor(
                    eq[:], x[:, ka:Kt, :],
                    m[:, ka:Kt, None].to_broadcast([128, kd, NCLS]), ALU.is_ge,
                )
                nc.vector.tensor_tensor(
                    s[:, ka:Kt, :], eq[:],
                    rev_sb[:, None, :].to_broadcast([128, kd, NCLS]), ALU.mult,
                )

            # bf16 halving tree (2x mode) then a small grouped reduce.
            t1 = tp.tile([128, Kt, 40], BF16, tag="t1")
            nc.vector.tensor_tensor(t1[:], s[:, :, 0:40], s[:, :, 40:80], ALU.max)
            t2 = tp.tile([128, Kt, 20], BF16, tag="t2")
            nc.vector.tensor_tensor(t2[:], t1[:, :, 0:20], t1[:, :, 20:40], ALU.max)
            t3 = tp.tile([128, Kt, 10], BF16, tag="t3")
            nc.vector.tensor_tensor(t3[:], t2[:, :, 0:10], t2[:, :, 10:20], ALU.max)
            mx = smalls.tile([128, Kt], BF16, tag="mx")
            nc.vector.reduce_max(mx[:], t3[:], axis=AXX)

            idx = smalls.tile([128, Kt], I32, tag="idx")
            # idx = 79 - 128*mx  ==  (mx * -128) + 79
            nc.vector.tensor_scalar(idx[:], mx[:], -128.0, 79.0, op0=ALU.mult, op1=ALU.add)
            nc.sync.dma_start(out=idx_vt, in_=idx[:])

            pend_conf.append((mc, conf_vt, Kt, t))

            if t == 2:
                emit_box_consts()
            if t == 3:
                emit_boxes()

            # Flush sigmoid/mult/sqrt/DMA in batches so ACT table loads stay
            # batched (Sign runs dominate the tile loop) and conf DMAs don't
            # all land in the tail.
            if t in (len(KS) - 3, len(KS) - 1):
                for mc_p, conf_p, Kp, tp_ in pend_conf:
                    sig = smalls.tile([128, 2 * Kp], F32, tag="sig")
                    nc.scalar.activation(sig[:], mc_p[:], AF.Sigmoid)
                    prod = prodp.tile([128, Kp], F32, tag=f"prod{tp_}")
                    nc.vector.tensor_tensor(
                        prod[:], sig[:, 0:Kp], sig[:, Kp : 2 * Kp], ALU.mult
                    )
                    pend_sqrt.append((prod, conf_p, Kp))
                for prod_p, conf_p, Kp in pend_sqrt:
                    conf = smalls.tile([128, Kp], F32, tag="conf")
                    nc.scalar.activation(conf[:], prod_p[:], AF.Sqrt)
                    nc.sync.dma_start(out=conf_p, in_=conf[:])
                pend_conf = []
                pend_sqrt = []

    nc.compile()
    return nc


def _get_program():
    if "nc" not in _prog_cache:
        _prog_cache["nc"] = _build_program()
    return _prog_cache["nc"]


def _make_in_maps(t_ltrb, center_logits, cls_logits):
    in_maps = []
    for i in range(N_CORES):
        b0, b1 = i * B_PER, (i + 1) * B_PER
        in_maps.append(
            {
                "cls": np.ascontiguousarray(
                    cls_logits[b0:b1].reshape(CELLS, NCLS), dtype=np.float32
                ),
                "ctr": np.ascontiguousarray(
                    center_logits[b0:b1].reshape(CELLS), dtype=np.float32
                ),
                "ltrb": np.ascontiguousarray(t_ltrb[b0:b1], dtype=np.float32),
            }
        )
    return in_maps


def kernel(t_ltrb, center_logits, cls_logits, img_h=None, img_w=None, **_unused):
    t_ltrb = np.asarray(t_ltrb)
    center_logits = np.asarray(center_logits)
    cls_logits = np.asarray(cls_logits)
    nB = t_ltrb.shape[0]
    assert nB == N_CORES * B_PER

    nc = _get_program()
    in_maps = _make_in_maps(t_ltrb, center_logits, cls_logits)
    trace = bool(int(os.environ.get("KERNEL_TRACE", "0")))
    res = run_bass_kernel_spmd(nc, in_maps, core_ids=list(range(N_CORES)), trace=trace)
    _prog_cache["last_exec_time_ns"] = res.exec_time_ns
    _prog_cache["last_trace"] = res.instructions_and_trace

    xywh = np.concatenate([r["xywh"].reshape(B_PER, H * W, 4) for r in res.results], axis=0)
    cls_idx = np.concatenate([r["idx"].reshape(B_PER, H * W) for r in res.results], axis=0)
    confs = np.concatenate([r["conf"].reshape(B_PER, H * W) for r in res.results], axis=0)
    return xywh, cls_idx.astype(np.int32), confs


# revision 36
# speedup vs baseline: 1.0066x; 1.0066x over previous
"""FCOS detection-head decode kernel for Trainium2 (Bass/Tile), 8-core data-parallel.

Problem: nB=32, nH=nW=128, nCls=80, stride=8.
  p_ltrb  = exp(t_ltrb) * 8
  p_xywh  = ltrb->xywh against the 8-px grid
  cls_idx = argmax_c cls_logits  (== argmax of sigmoid(cls_logits), sigmoid monotone;
            verified bit-exact vs the reference on the fixed dataset incl. ties)
  confs   = sqrt(sigmoid(center) * sigmoid(max_logit))

Sharding: batch dim 32 -> 4 batches per core, 8 cores SPMD (no collectives).

Per-core pipeline (cells = 4*128*128 = 65536, cell-major layout, K cells/partition):
  DVE : m   = grouped reduce_max over classes            [128, K, 80] -> [128, K]
  DVE : eq  = (x >= m) as bf16 0/1                       (TT is_ge, broadcast m)
  DVE : s   = eq * rev (rev = 79-c, bf16, 2x mode)
  DVE : mx  = grouped max(s) via bf16 2x halving tree    -> idx = 79-mx (first occurrence)
  ACT : sigmoid(m), sigmoid(center), sqrt; exp for boxes
"""

import os
import numpy as np
import ml_dtypes

import concourse.bass as bass
import concourse.bacc as bacc
import concourse.tile as tile
from concourse import mybir
from concourse.bass_utils import run_bass_kernel_spmd

F32 = mybir.dt.float32
BF16 = mybir.dt.bfloat16
I32 = mybir.dt.int32
ALU = mybir.AluOpType
AF = mybir.ActivationFunctionType
AXX = mybir.AxisListType.X

N_CORES = 8
B_PER = 4                      # batches per core
H = W = 128
NCLS = 80
CELLS = B_PER * H * W          # 65536 cells per core
K = 64                         # cells per partition per cls tile
TCELLS = 128 * K               # cells per tile
NT = CELLS // TCELLS           # number of cls tiles
STRIDE = 8.0
LN4 = float(np.log(4.0))
LN8 = float(np.log(8.0))
KACT = 44  # k-slices per full tile whose compare runs on ACT (Sign) instead of DVE

_prog_cache = {}


def _build_program():
    nc = bacc.Bacc("TRN2", debug=False)

    cls_t = nc.dram_tensor("cls", [CELLS, NCLS], F32, kind="ExternalInput")
    ctr_t = nc.dram_tensor("ctr", [CELLS], F32, kind="ExternalInput")
    ltrb_t = nc.dram_tensor("ltrb", [B_PER, H, W, 4], F32, kind="ExternalInput")
    xywh_t = nc.dram_tensor("xywh", [B_PER, H, W, 4], F32, kind="ExternalOutput")
    idx_t = nc.dram_tensor("idx", [CELLS], I32, kind="ExternalOutput")
    conf_t = nc.dram_tensor("conf", [CELLS], F32, kind="ExternalOutput")

    # Constants embedded in the NEFF. rev = (79-c)/128: exact in bf16 (k/128,
    # k < 128), so every value in the extract chain stays exact.
    rev_np = np.broadcast_to(
        ((79 - np.arange(NCLS)) / 128.0).astype(ml_dtypes.bfloat16), (128, NCLS)
    )
    xg_np = np.broadcast_to((np.arange(W) * STRIDE + STRIDE / 2).astype(np.float32), (128, W))
    yg_np = (np.arange(128) * STRIDE + STRIDE / 2).astype(np.float32).reshape(128, 1)
    rev_d = nc.inline_tensor(np.ascontiguousarray(rev_np), "rev_const")
    xg_d = nc.inline_tensor(np.ascontiguousarray(xg_np), "xg_const")
    yg_d = nc.inline_tensor(yg_np, "yg_const")

    from contextlib import ExitStack

    with ExitStack() as ctx:
        tc = ctx.enter_context(tile.TileContext(nc))
        consts = ctx.enter_context(tc.tile_pool(name="consts", bufs=1))
        xp = ctx.enter_context(tc.tile_pool(name="xp", bufs=3))
        eqp = ctx.enter_context(tc.tile_pool(name="eqp", bufs=2))
        sp = ctx.enter_context(tc.tile_pool(name="sp", bufs=3))
        zp = ctx.enter_context(tc.tile_pool(name="zp", bufs=3))
        tp = ctx.enter_context(tc.tile_pool(name="tp", bufs=2))
        smalls = ctx.enter_context(tc.tile_pool(name="smalls", bufs=3))
        prodp = ctx.enter_context(tc.tile_pool(name="prodp", bufs=1))
        boxp = ctx.enter_context(tc.tile_pool(name="boxp", bufs=1))

        rev_sb = consts.tile([128, NCLS], BF16)
        nc.sync.dma_start(out=rev_sb[:], in_=rev_d.ap())
        xg_sb = consts.tile([128, W], F32)
        yg_sb = consts.tile([128, 1], F32)
        ln4_sb = consts.tile([128, 1], F32)
        ln8_sb = consts.tile([128, 1], F32)

        def emit_box_consts():
            nc.sync.dma_start(out=xg_sb[:], in_=xg_d.ap())
            nc.sync.dma_start(out=yg_sb[:], in_=yg_d.ap())
            nc.vector.memset(ln4_sb[:], LN4)
            nc.vector.memset(ln8_sb[:], LN8)

        cls_f = cls_t.ap()
        ctr_f = ctr_t.ap()
        idx_f = idx_t.ap()
        conf_f = conf_t.ap()

        def emit_boxes():
            # Box path: one tile for all batches, [128 part = h, (b, w, 4)] layout.
            lt = boxp.tile([128, B_PER, W, 4], F32, tag="lt")
            nc.sync.dma_start(
                out=lt[:], in_=ltrb_t.ap().rearrange("b h w c -> h b w c")
            )
            a4 = boxp.tile([128, B_PER, W, 4], F32, tag="a4")
            nc.scalar.activation(a4[:], lt[:], AF.Exp, bias=ln4_sb[:])  # 4*exp(t)
            a8 = boxp.tile([128, B_PER, W, 4], F32, tag="a8")
            nc.scalar.activation(a8[:], lt[:], AF.Exp, bias=ln8_sb[:])  # 8*exp(t)

            xy = boxp.tile([128, B_PER, W, 4], F32, tag="xy")
            tmp = boxp.tile([128, B_PER, W], F32, tag="btmp")
            # cx = xg + (a4_r - a4_l)
            nc.vector.tensor_tensor(tmp[:], a4[:, :, :, 2], a4[:, :, :, 0], ALU.subtract)
            nc.vector.tensor_tensor(
                xy[:, :, :, 0], tmp[:],
                xg_sb[:, None, :].to_broadcast([128, B_PER, W]), ALU.add,
            )
            # cy = (a4_b + yg) - a4_t   (one fused scalar_tensor_tensor)
            nc.vector.scalar_tensor_tensor(
                xy[:, :, :, 1], a4[:, :, :, 3], yg_sb[:], a4[:, :, :, 1],
                op0=ALU.add, op1=ALU.subtract,
            )
            # w = a8_l + a8_r ; h = a8_t + a8_b
            nc.vector.tensor_tensor(xy[:, :, :, 2], a8[:, :, :, 0], a8[:, :, :, 2], ALU.add)
            nc.vector.tensor_tensor(xy[:, :, :, 3], a8[:, :, :, 1], a8[:, :, :, 3], ALU.add)
            nc.sync.dma_start(
                out=xywh_t.ap().rearrange("b h w c -> h b w c"), in_=xy[:]
            )

        # Ramp-up schedule: small first tiles so DVE starts quickly, then
        # full-size tiles. Sum of Ks must equal CELLS // 128.
        KS = [8, 8, 16, 32] + [K] * ((CELLS // 128 - 128) // K) + [32, 32]
        assert sum(KS) == CELLS // 128

        pend_conf = []
        pend_sqrt = []
        c0 = 0
        for t, Kt in enumerate(KS):
            n = 128 * Kt
            cls_vt = cls_f[c0 : c0 + n].rearrange("(p k) c -> p k c", k=Kt)
            ctr_vt = ctr_f[c0 : c0 + n].rearrange("(p k) -> p k", k=Kt)
            idx_vt = idx_f[c0 : c0 + n].rearrange("(p k) -> p k", k=Kt)
            conf_vt = conf_f[c0 : c0 + n].rearrange("(p k) -> p k", k=Kt)
            c0 += n

            x = xp.tile([128, Kt, NCLS], F32, tag="x")
            nc.sync.dma_start(out=x[:], in_=cls_vt)

            # m and center logits share one tile so one Sigmoid covers both.
            # Per-tile tag: mc stays alive until the batched sigmoid flush.
            mc = prodp.tile([128, 2 * Kt], F32, tag=f"mc{t}")
            m = mc[:, 0:Kt]
            nc.vector.reduce_max(m, x[:], axis=AXX)
            nc.sync.dma_start(out=mc[:, Kt : 2 * Kt], in_=ctr_vt)

            s = sp.tile([128, Kt, NCLS], BF16, tag="s")

            # Split the compare between ACT (Sign with per-partition -m bias,
            # one k-slice per instruction) and DVE (is_ge TT). ACT is idle;
            # the ACT share's masked-rev product fuses into one 4x-mode STT.
            ka = KACT if Kt == K else 0
            if ka:
                mneg = smalls.tile([128, Kt], F32, tag="mneg")
                nc.vector.tensor_scalar(mneg[:], m, -1.0, None, op0=ALU.mult)
                z = zp.tile([128, ka, NCLS], BF16, tag="z")
                for kk in range(ka):
                    # z = sign(x - m) in {-1, 0}
                    nc.scalar.sign(z[:, kk, :], x[:, kk, :], bias=mneg[:, kk : kk + 1])
                # s = z + rev: achiever -> rev >= 0, else rev-1 < 0 (all exact bf16)
                nc.vector.tensor_tensor(
                    s[:, 0:ka, :], z[:],
                    rev_sb[:, None, :].to_broadcast([128, ka, NCLS]), ALU.add,
                )

            kd = Kt - ka
            if kd:
                eq = eqp.tile([128, kd, NCLS], BF16, tag="eq")
                nc.vector.tensor_tensor(
                    eq[:], x[:, ka:Kt, :],
                    m[:, ka:Kt, None].to_broadcast([128, kd, NCLS]), ALU.is_ge,
                )
                nc.vector.tensor_tensor(
                    s[:, ka:Kt, :], eq[:],
                    rev_sb[:, None, :].to_broadcast([128, kd, NCLS]), ALU.mult,
                )

            # bf16 halving tree (2x mode) then a small grouped reduce.
            t1 = tp.tile([128, Kt, 40], BF16, tag="t1")
            nc.vector.tensor_tensor(t1[:], s[:, :, 0:40], s[:, :, 40:80], ALU.max)
            t2 = tp.tile([128, Kt, 20], BF16, tag="t2")
            nc.vector.tensor_tensor(t2[:], t1[:, :, 0:20], t1[:, :, 20:40], ALU.max)
            t3 = tp.tile([128, Kt, 10], BF16, tag="t3")
            nc.vector.tensor_tensor(t3[:], t2[:, :, 0:10], t2[:, :, 10:20], ALU.max)
            mx = smalls.tile([128, Kt], BF16, tag="mx")
            nc.vector.reduce_max(mx[:], t3[:], axis=AXX)

            idx = smalls.tile([128, Kt], I32, tag="idx")
            # idx = 79 - 128*mx  ==  (mx * -128) + 79
            nc.vector.tensor_scalar(idx[:], mx[:], -128.0, 79.0, op0=ALU.mult, op1=ALU.add)
            nc.sync.dma_start(out=idx_vt, in_=idx[:])

            pend_conf.append((mc, conf_vt, Kt, t))

            if t == 2:
                emit_box_consts()
            if t == 3:
                emit_boxes()

            # Flush sigmoid/mult/sqrt/DMA in batches so ACT table loads stay
            # batched (Sign runs dominate the tile loop) and conf DMAs don't
            # all land in the tail.
            if t in (len(KS) - 3, len(KS) - 1):
                for mc_p, conf_p, Kp, tp_ in pend_conf:
                    sig = smalls.tile([128, 2 * Kp], F32, tag="sig")
                    nc.scalar.activation(sig[:], mc_p[:], AF.Sigmoid)
                    prod = prodp.tile([128, Kp], F32, tag=f"prod{tp_}")
                    nc.vector.tensor_tensor(
                        prod[:], sig[:, 0:Kp], sig[:, Kp : 2 * Kp], ALU.mult
                    )
                    pend_sqrt.append((prod, conf_p, Kp))
                for prod_p, conf_p, Kp in pend_sqrt:
                    conf = smalls.tile([128, Kp], F32, tag="conf")
                    nc.scalar.activation(conf[:], prod_p[:], AF.Sqrt)
                    nc.sync.dma_start(out=conf_p, in_=conf[:])
                pend_conf = []
                pend_sqrt = []

    nc.compile()
    return nc


def _get_program():
    if "nc" not in _prog_cache:
        _prog_cache["nc"] = _build_program()
    return _prog_cache["nc"]


def _make_in_maps(t_ltrb, center_logits, cls_logits):
    in_maps = []
    for i in range(N_CORES):
        b0, b1 = i * B_PER, (i + 1) * B_PER
        in_maps.append(
            {
                "cls": np.ascontiguousarray(
                    cls_logits[b0:b1].reshape(CELLS, NCLS), dtype=np.float32
                ),
                "ctr": np.ascontiguousarray(
                    center_logits[b0:b1].reshape(CELLS), dtype=np.float32
                ),
                "ltrb": np.ascontiguousarray(t_ltrb[b0:b1], dtype=np.float32),
            }
        )
    return in_maps


def kernel(t_ltrb, center_logits, cls_logits, img_h=None, img_w=None, **_unused):
    t_ltrb = np.asarray(t_ltrb)
    center_logits = np.asarray(center_logits)
    cls_logits = np.asarray(cls_logits)
    nB = t_ltrb.shape[0]
    assert nB == N_CORES * B_PER

    nc = _get_program()
    in_maps = _make_in_maps(t_ltrb, center_logits, cls_logits)
    trace = bool(int(os.environ.get("KERNEL_TRACE", "0")))
    res = run_bass_kernel_spmd(nc, in_maps, core_ids=list(range(N_CORES)), trace=trace)
    _prog_cache["last_exec_time_ns"] = res.exec_time_ns
    _prog_cache["last_trace"] = res.instructions_and_trace

    xywh = np.concatenate([r["xywh"].reshape(B_PER, H * W, 4) for r in res.results], axis=0)
    cls_idx = np.concatenate([r["idx"].reshape(B_PER, H * W) for r in res.results], axis=0)
    confs = np.concatenate([r["conf"].reshape(B_PER, H * W) for r in res.results], axis=0)
    return xywh, cls_idx.astype(np.int32), confs


# revision 39
# speedup vs baseline: 1.0250x; 1.0183x over previous
"""FCOS detection-head decode kernel for Trainium2 (Bass/Tile), 8-core data-parallel.

Problem: nB=32, nH=nW=128, nCls=80, stride=8.
  p_ltrb  = exp(t_ltrb) * 8
  p_xywh  = ltrb->xywh against the 8-px grid
  cls_idx = argmax_c cls_logits  (== argmax of sigmoid(cls_logits), sigmoid monotone;
            verified bit-exact vs the reference on the fixed dataset incl. ties)
  confs   = sqrt(sigmoid(center) * sigmoid(max_logit))

Sharding: batch dim 32 -> 4 batches per core, 8 cores SPMD (no collectives).

Per-core pipeline (cells = 4*128*128 = 65536, cell-major layout, K cells/partition):
  DVE : m   = grouped reduce_max over classes            [128, K, 80] -> [128, K]
  DVE : eq  = (x >= m) as bf16 0/1                       (TT is_ge, broadcast m)
  DVE : s   = eq * rev (rev = 79-c, bf16, 2x mode)
  DVE : mx  = grouped max(s) via bf16 2x halving tree    -> idx = 79-mx (first occurrence)
  ACT : sigmoid(m), sigmoid(center), sqrt; exp for boxes
"""

import os
import numpy as np
import ml_dtypes

import concourse.bass as bass
import concourse.bacc as bacc
import concourse.tile as tile
from concourse import mybir
from concourse.bass_utils import run_bass_kernel_spmd

F32 = mybir.dt.float32
BF16 = mybir.dt.bfloat16
I32 = mybir.dt.int32
ALU = mybir.AluOpType
AF = mybir.ActivationFunctionType
AXX = mybir.AxisListType.X

N_CORES = 8
B_PER = 4                      # batches per core
H = W = 128
NCLS = 80
CELLS = B_PER * H * W          # 65536 cells per core
K = 64                         # cells per partition per cls tile
TCELLS = 128 * K               # cells per tile
NT = CELLS // TCELLS           # number of cls tiles
STRIDE = 8.0
LN4 = float(np.log(4.0))
LN8 = float(np.log(8.0))
KACT = 48  # k-slices per full tile whose compare runs on ACT (Sign) instead of DVE

_prog_cache = {}


def _build_program():
    nc = bacc.Bacc("TRN2", debug=False)

    cls_t = nc.dram_tensor("cls", [CELLS, NCLS], F32, kind="ExternalInput")
    ctr_t = nc.dram_tensor("ctr", [CELLS], F32, kind="ExternalInput")
    ltrb_t = nc.dram_tensor("ltrb", [B_PER, H, W, 4], F32, kind="ExternalInput")
    xywh_t = nc.dram_tensor("xywh", [B_PER, H, W, 4], F32, kind="ExternalOutput")
    idx_t = nc.dram_tensor("idx", [CELLS], I32, kind="ExternalOutput")
    conf_t = nc.dram_tensor("conf", [CELLS], F32, kind="ExternalOutput")

    # Constants embedded in the NEFF. rev = (79-c)/128: exact in bf16 (k/128,
    # k < 128), so every value in the extract chain stays exact.
    rev_np = np.broadcast_to(
        ((79 - np.arange(NCLS)) / 128.0).astype(ml_dtypes.bfloat16), (128, NCLS)
    )
    xg_np = np.broadcast_to((np.arange(W) * STRIDE + STRIDE / 2).astype(np.float32), (128, W))
    yg_np = (np.arange(128) * STRIDE + STRIDE / 2).astype(np.float32).reshape(128, 1)
    rev_d = nc.inline_tensor(np.ascontiguousarray(rev_np), "rev_const")
    xg_d = nc.inline_tensor(np.ascontiguousarray(xg_np), "xg_const")
    yg_d = nc.inline_tensor(yg_np, "yg_const")

    from contextlib import ExitStack

    with ExitStack() as ctx:
        tc = ctx.enter_context(tile.TileContext(nc))
        consts = ctx.enter_context(tc.tile_pool(name="consts", bufs=1))
        xp = ctx.enter_context(tc.tile_pool(name="xp", bufs=3))
        eqp = ctx.enter_context(tc.tile_pool(name="eqp", bufs=2))
        sp = ctx.enter_context(tc.tile_pool(name="sp", bufs=3))
        zp = ctx.enter_context(tc.tile_pool(name="zp", bufs=3))
        tp = ctx.enter_context(tc.tile_pool(name="tp", bufs=2))
        smalls = ctx.enter_context(tc.tile_pool(name="smalls", bufs=3))
        prodp = ctx.enter_context(tc.tile_pool(name="prodp", bufs=1))
        boxp = ctx.enter_context(tc.tile_pool(name="boxp", bufs=1))

        rev_sb = consts.tile([128, NCLS], BF16)
        nc.sync.dma_start(out=rev_sb[:], in_=rev_d.ap())
        xg_sb = consts.tile([128, W], F32)
        yg_sb = consts.tile([128, 1], F32)
        ln4_sb = consts.tile([128, 1], F32)
        ln8_sb = consts.tile([128, 1], F32)

        def emit_box_consts():
            nc.sync.dma_start(out=xg_sb[:], in_=xg_d.ap())
            nc.sync.dma_start(out=yg_sb[:], in_=yg_d.ap())
            nc.vector.memset(ln4_sb[:], LN4)
            nc.vector.memset(ln8_sb[:], LN8)

        cls_f = cls_t.ap()
        ctr_f = ctr_t.ap()
        idx_f = idx_t.ap()
        conf_f = conf_t.ap()

        def emit_boxes():
            # Box path: one tile for all batches, [128 part = h, (b, w, 4)] layout.
            lt = boxp.tile([128, B_PER, W, 4], F32, tag="lt")
            nc.sync.dma_start(
                out=lt[:], in_=ltrb_t.ap().rearrange("b h w c -> h b w c")
            )
            a4 = boxp.tile([128, B_PER, W, 4], F32, tag="a4")
            nc.scalar.activation(a4[:], lt[:], AF.Exp, bias=ln4_sb[:])  # 4*exp(t)
            a8 = boxp.tile([128, B_PER, W, 4], F32, tag="a8")
            nc.scalar.activation(a8[:], lt[:], AF.Exp, bias=ln8_sb[:])  # 8*exp(t)

            xy = boxp.tile([128, B_PER, W, 4], F32, tag="xy")
            tmp = boxp.tile([128, B_PER, W], F32, tag="btmp")
            # cx = xg + (a4_r - a4_l)
            nc.vector.tensor_tensor(tmp[:], a4[:, :, :, 2], a4[:, :, :, 0], ALU.subtract)
            nc.vector.tensor_tensor(
                xy[:, :, :, 0], tmp[:],
                xg_sb[:, None, :].to_broadcast([128, B_PER, W]), ALU.add,
            )
            # cy = (a4_b + yg) - a4_t   (one fused scalar_tensor_tensor)
            nc.vector.scalar_tensor_tensor(
                xy[:, :, :, 1], a4[:, :, :, 3], yg_sb[:], a4[:, :, :, 1],
                op0=ALU.add, op1=ALU.subtract,
            )
            # w = a8_l + a8_r ; h = a8_t + a8_b
            nc.vector.tensor_tensor(xy[:, :, :, 2], a8[:, :, :, 0], a8[:, :, :, 2], ALU.add)
            nc.vector.tensor_tensor(xy[:, :, :, 3], a8[:, :, :, 1], a8[:, :, :, 3], ALU.add)
            nc.sync.dma_start(
                out=xywh_t.ap().rearrange("b h w c -> h b w c"), in_=xy[:]
            )

        # Ramp-up schedule: small first tiles so DVE starts quickly, then
        # full-size tiles. Sum of Ks must equal CELLS // 128.
        KS = [8, 8, 16, 32] + [K] * ((CELLS // 128 - 128) // K) + [32, 32]
        assert sum(KS) == CELLS // 128

        pend_conf = []
        pend_sqrt = []
        c0 = 0
        for t, Kt in enumerate(KS):
            n = 128 * Kt
            cls_vt = cls_f[c0 : c0 + n].rearrange("(p k) c -> p k c", k=Kt)
            ctr_vt = ctr_f[c0 : c0 + n].rearrange("(p k) -> p k", k=Kt)
            idx_vt = idx_f[c0 : c0 + n].rearrange("(p k) -> p k", k=Kt)
            conf_vt = conf_f[c0 : c0 + n].rearrange("(p k) -> p k", k=Kt)
            c0 += n

            x = xp.tile([128, Kt, NCLS], F32, tag="x")
            nc.sync.dma_start(out=x[:], in_=cls_vt)

            # m and center logits share one tile so one Sigmoid covers both.
            # Per-tile tag: mc stays alive until the batched sigmoid flush.
            mc = prodp.tile([128, 2 * Kt], F32, tag=f"mc{t}")
            m = mc[:, 0:Kt]
            nc.vector.reduce_max(m, x[:], axis=AXX)
            nc.sync.dma_start(out=mc[:, Kt : 2 * Kt], in_=ctr_vt)

            s = sp.tile([128, Kt, NCLS], BF16, tag="s")

            # Split the compare between ACT (Sign with per-partition -m bias,
            # one k-slice per instruction) and DVE (is_ge TT). ACT is idle;
            # the ACT share's masked-rev product fuses into one 4x-mode STT.
            ka = KACT if Kt == K else 0
            if ka:
                mneg = smalls.tile([128, Kt], F32, tag="mneg")
                nc.vector.tensor_scalar(mneg[:], m, -1.0, None, op0=ALU.mult)
                z = zp.tile([128, ka, NCLS], BF16, tag="z")
                for kk in range(ka):
                    # z = sign(x - m) in {-1, 0}
                    nc.scalar.sign(z[:, kk, :], x[:, kk, :], bias=mneg[:, kk : kk + 1])
                # s = z + rev: achiever -> rev >= 0, else rev-1 < 0 (all exact bf16)
                nc.vector.tensor_tensor(
                    s[:, 0:ka, :], z[:],
                    rev_sb[:, None, :].to_broadcast([128, ka, NCLS]), ALU.add,
                )

            kd = Kt - ka
            if kd:
                eq = eqp.tile([128, kd, NCLS], BF16, tag="eq")
                nc.vector.tensor_tensor(
                    eq[:], x[:, ka:Kt, :],
                    m[:, ka:Kt, None].to_broadcast([128, kd, NCLS]), ALU.is_ge,
                )
                nc.vector.tensor_tensor(
                    s[:, ka:Kt, :], eq[:],
                    rev_sb[:, None, :].to_broadcast([128, kd, NCLS]), ALU.mult,
                )

            # bf16 halving tree (2x mode) then a small grouped reduce.
            t1 = tp.tile([128, Kt, 40], BF16, tag="t1")
            nc.vector.tensor_tensor(t1[:], s[:, :, 0:40], s[:, :, 40:80], ALU.max)
            t2 = tp.tile([128, Kt, 20], BF16, tag="t2")
            nc.vector.tensor_tensor(t2[:], t1[:, :, 0:20], t1[:, :, 20:40], ALU.max)
            t3 = tp.tile([128, Kt, 10], BF16, tag="t3")
            nc.vector.tensor_tensor(t3[:], t2[:, :, 0:10], t2[:, :, 10:20], ALU.max)
            mx = smalls.tile([128, Kt], BF16, tag="mx")
            nc.vector.reduce_max(mx[:], t3[:], axis=AXX)

            idx = smalls.tile([128, Kt], I32, tag="idx")
            # idx = 79 - 128*mx  ==  (mx * -128) + 79
            nc.vector.tensor_scalar(idx[:], mx[:], -128.0, 79.0, op0=ALU.mult, op1=ALU.add)
            nc.sync.dma_start(out=idx_vt, in_=idx[:])

            pend_conf.append((mc, conf_vt, Kt, t))

            if t == 2:
                emit_box_consts()

            # Flush sigmoid/mult/sqrt/DMA in batches so ACT table loads stay
            # batched (Sign runs dominate the tile loop) and conf DMAs don't
            # all land in the tail.
            if t in (len(KS) - 3, len(KS) - 1):
                for mc_p, conf_p, Kp, tp_ in pend_conf:
                    sig = smalls.tile([128, 2 * Kp], F32, tag="sig")
                    nc.scalar.activation(sig[:], mc_p[:], AF.Sigmoid)
                    prod = prodp.tile([128, Kp], F32, tag=f"prod{tp_}")
                    nc.vector.tensor_tensor(
                        prod[:], sig[:, 0:Kp], sig[:, Kp : 2 * Kp], ALU.mult
                    )
                    pend_sqrt.append((prod, conf_p, Kp))
                for prod_p, conf_p, Kp in pend_sqrt:
                    conf = smalls.tile([128, Kp], F32, tag="conf")
                    nc.scalar.activation(conf[:], prod_p[:], AF.Sqrt)
                    nc.sync.dma_start(out=conf_p, in_=conf[:])
                pend_conf = []
                pend_sqrt = []
                # Boxes ride the flush's ACT-table transition window (the Exp
                # loads happen while tables are being swapped anyway).
                if t == len(KS) - 3:
                    emit_boxes()

    nc.compile()
    return nc


def _get_program():
    if "nc" not in _prog_cache:
        _prog_cache["nc"] = _build_program()
    return _prog_cache["nc"]


def _make_in_maps(t_ltrb, center_logits, cls_logits):
    in_maps = []
    for i in range(N_CORES):
        b0, b1 = i * B_PER, (i + 1) * B_PER
        in_maps.append(
            {
                "cls": np.ascontiguousarray(
                    cls_logits[b0:b1].reshape(CELLS, NCLS), dtype=np.float32
                ),
                "ctr": np.ascontiguousarray(
                    center_logits[b0:b1].reshape(CELLS), dtype=np.float32
                ),
                "ltrb": np.ascontiguousarray(t_ltrb[b0:b1], dtype=np.float32),
            }
        )
    return in_maps


def kernel(t_ltrb, center_logits, cls_logits, img_h=None, img_w=None, **_unused):
    t_ltrb = np.asarray(t_ltrb)
    center_logits = np.asarray(center_logits)
    cls_logits = np.asarray(cls_logits)
    nB = t_ltrb.shape[0]
    assert nB == N_CORES * B_PER

    nc = _get_program()
    in_maps = _make_in_maps(t_ltrb, center_logits, cls_logits)
    trace = bool(int(os.environ.get("KERNEL_TRACE", "0")))
    res = run_bass_kernel_spmd(nc, in_maps, core_ids=list(range(N_CORES)), trace=trace)
    _prog_cache["last_exec_time_ns"] = res.exec_time_ns
    _prog_cache["last_trace"] = res.instructions_and_trace

    xywh = np.concatenate([r["xywh"].reshape(B_PER, H * W, 4) for r in res.results], axis=0)
    cls_idx = np.concatenate([r["idx"].reshape(B_PER, H * W) for r in res.results], axis=0)
    confs = np.concatenate([r["conf"].reshape(B_PER, H * W) for r in res.results], axis=0)
    return xywh, cls_idx.astype(np.int32), confs
